# revision 7
# baseline (speedup 1.0000x reference)
"""Trainium2 Bass kernel v2: BinaryHungarianMatcherV2 cost-matrix build.

C[b,q,t] = 5*L1(pred_box, tgt_box) + 2*focal_class(q) + 2 - 2*giou,
invalid targets (t >= num_boxes[b]) fixed to 1e9 on the host.

Layout: t on the partition axis, q on the free axis (1800 wide). Per core
4 batch slots (batch dim sharded over 8 cores, slots sorted by num_boxes
so cores sharing the SPMD program do similar work); per slot
ceil(W/128) t-slabs of [128 x 1800]. Per-target values ride as
per-partition scalar columns; per-query values are bf16 streams
replicated across partitions (one DMA per slot, triple-buffered so
prefetch never waits on consumers).

Work is split across three engines per slab (bf16, DVE 2x mode for
tensor_tensor), software-pipelined with the DVE running phases
A(K) | C2(K-2) | C1(K-1) and the Pool running we/he one slab ahead of
its u1/out tail so no engine's in-order queue blocks another's inputs:
  DVE  A : wd/hd corner-overlap customs, -2*inter custom,
           tensor_scalar (a2 - inter), union add
  ACT    : tw/th (we/he partials), r1 = 1/union, 4x Abs (L1 terms),
           r2(K-1) = -2/areae (one slab late by construction)
  Pool   : we/he adds (slab K+1), u1 = s12 + cc2, out = u1 + g
  DVE C1 : areae = we*he, s1/s2/s12 abs-sums
  DVE C2 : p1 = inter2*r1, p2 = union*r2, g = p1+p2
Intermediate tiles alias where lifetimes are disjoint (p1/p2/g reuse the
abs tiles, areae/s2 reuse wd/hd, r2 reuses we).
"""

import os
from contextlib import ExitStack

import numpy as np

B, Q, T = 32, 1800, 500
N_CORES = 8
B_PER = B // N_CORES
TP = 128                       # t-partition tile size
NSTR = 5                       # streams: cx, w, cy, h, a1 (wd's pair first)
S_CX, S_W, S_CY, S_H, S_A1 = range(NSTR)
NKC = 11                       # per-slab scalar columns
K_X0, K_X1, K_Y0, K_Y1, K_BCX, K_BCY, K_BW, K_BH, K_WT, K_HT, K_A2 = range(NKC)

INVALID = 1.0e9

_OPS = None
_PROG_CACHE = {}
LAST_RESULTS = None


def _get_ops():
    """Register custom DVE ops (idempotent)."""
    global _OPS
    if _OPS is not None:
        return _OPS
    from concourse import dve_ops
    from concourse.dve_ops import DveOp
    from concourse.dve_spec import Spec, Src0, Src1, C0, C1, C2, relu, maxx, minn, lower
    from concourse.dve_uop import DveOpSpec

    def reg(name, spec):
        for op in dve_ops.OPS:
            if op.name == name:
                return op
        row = max(dve_ops._SUB_OPCODE_FOR_NAME.values()) + 1
        assert row < 0x20, "custom-DVE opcode rows exhausted"
        dve_ops._SUB_OPCODE_FOR_NAME[name] = row
        shas = {}
        for ver in ("v3", "v4"):
            s = DveOpSpec(name=name, opcode=row, uops=lower(spec, ver=ver),
                          rd1_en=dve_ops.has_src1(spec))
            shas[ver] = s.sha(ver)
        op = DveOp(name, spec, subdim=False, uops_sha=shas)
        dve_ops.OPS.append(op)
        dve_ops.CUSTOM_DVE_SPECS[name] = spec
        return op

    _OPS = {
        # wd = min(cx + 0.5*w, x1t) - max(cx - 0.5*w, x0t); C0=x1t, C1=x0t, C2=0.5
        "BHM_IDIFFC": reg("BHM_IDIFFC", Spec(
            body=minn(Src0 + Src1 * C2, C0) - maxx(Src0 - Src1 * C2, C1),
            reference=lambda in0, in1, s0, s1, imm2:
                np.minimum(in0 + in1 * imm2, s0) - np.maximum(in0 - in1 * imm2, s1))),
        # inter2 = relu(wd)*relu(hd)*C2 (C2 = -2)
        "BHM_RELUMULN": reg("BHM_RELUMULN", Spec(
            body=(relu(Src0) * relu(Src1)) * C2,
            reference=lambda in0, in1, s0, s1, imm2:
                np.maximum(in0, 0) * np.maximum(in1, 0) * imm2)),
    }
    return _OPS


def _plan(num_boxes):
    """Sort batches by num_boxes; slot j holds sorted[8j:8j+8] (one per core).
    Returns (slots[B_PER][N_CORES], ntiles tuple)."""
    nb = np.asarray(num_boxes).astype(np.int64)
    order = np.argsort(nb, kind="stable")
    slots = order.reshape(B_PER, N_CORES)
    ntiles = tuple(int(-(-int(nb[slots[j]].max()) // TP)) for j in range(B_PER))
    return slots, ntiles


def _build_program(ntiles):
    import concourse.bass as bass
    from concourse import mybir

    ops = _get_ops()
    f32 = mybir.dt.float32
    bf16 = mybir.dt.bfloat16
    alu = mybir.AluOpType
    AFT = mybir.ActivationFunctionType
    nc = bass.Bass("TRN2")

    slabs = [(j, i) for j in range(B_PER) for i in range(ntiles[j])]
    NK = len(slabs)
    REPEAT = int(os.environ.get("BHM_REPEAT", "1"))
    NTOT = NK * REPEAT
    GTOT = B_PER * REPEAT
    first_slab = {}
    last_slab = {}
    for k, (j, i) in enumerate(slabs):
        first_slab.setdefault(j, k)
        last_slab[j] = k

    def glast(g):
        """Global K index of the last slab of global slot g."""
        return (g // B_PER) * NK + last_slab[g % B_PER]

    qstr = nc.dram_tensor("qstr", [B_PER, TP, NSTR * Q], bf16,
                          kind="ExternalInput").ap()
    kcol = nc.dram_tensor("kcol", [TP, NK * NKC], f32, kind="ExternalInput").ap()
    # two half-results per slab; the host adds them (plus the per-query
    # class cost, which never has to touch the device) during assembly.
    cout = nc.dram_tensor("C", [NK, 2, TP, Q], bf16, kind="ExternalOutput").ap()

    with ExitStack() as ctx:
        st = [ctx.enter_context(nc.sbuf_tensor(f"st_{p}", [TP, NSTR * Q], bf16))
              for p in range(3)]
        kc = ctx.enter_context(nc.sbuf_tensor("kc", [TP, NK * NKC], f32))

        tnames = ["wd", "hd", "acx", "acy", "aw", "ah", "s1c", "s2", "tw",
                  "th", "we", "he", "r1", "r2", "areae", "g"]
        tl = {n: [ctx.enter_context(nc.sbuf_tensor(f"t_{n}_{p}", [TP, Q], bf16))
                  for p in range(2)] for n in tnames}
        for n in ("inter2", "tuU"):
            tl[n] = [ctx.enter_context(nc.sbuf_tensor(f"t_{n}_{p}", [TP, Q], bf16))
                     for p in range(3)]
        # disjoint-lifetime aliases whose safety the existing waits already
        # imply: abs(K+2) on ACT runs after the hoisted sG wait (C2(K) done,
        # so p1/p2(K) are dead).
        tl["p1"] = tl["acy"]
        tl["p2"] = tl["aw"]

        sINA = ctx.enter_context(nc.semaphore("sINA"))     # kcol + cx/w streams
        sINC = ctx.enter_context(nc.semaphore("sINC"))     # cy/h streams
        sINB = ctx.enter_context(nc.semaphore("sINB"))     # a1 streams
        sWD = ctx.enter_context(nc.semaphore("sWD"))       # DVE wd done
        sWH = ctx.enter_context(nc.semaphore("sWH"))       # DVE wd,hd done
        sDVEa = ctx.enter_context(nc.semaphore("sDVEa"))   # DVE union done
        sS12 = ctx.enter_context(nc.semaphore("sS12"))     # DVE s12 done
        sAREA = ctx.enter_context(nc.semaphore("sAREA"))   # DVE areae done
        sG = ctx.enter_context(nc.semaphore("sG"))         # DVE g done
        sTW = ctx.enter_context(nc.semaphore("sTW"))       # ACT tw done
        sTWTH = ctx.enter_context(nc.semaphore("sTWTH"))   # ACT tw,th done
        sABS = ctx.enter_context(nc.semaphore("sABS"))     # ACT abs group done
        sR1 = ctx.enter_context(nc.semaphore("sR1"))
        sR2 = ctx.enter_context(nc.semaphore("sR2"))
        pWE = ctx.enter_context(nc.semaphore("pWE"))       # Pool we,he done
        pS12 = ctx.enter_context(nc.semaphore("pS12"))     # Pool s12 done
        sSTORE = ctx.enter_context(nc.semaphore("sSTORE"))
        block = ctx.enter_context(nc.Block())

        def S(g, s):
            return st[g % 3][:, s * Q:(s + 1) * Q]

        def load_slot(sync, g):
            # wd's pair (cx,w) first, then hd's (cy,h), then a1, so the DVE
            # starts as soon as the first 0.9 MB lands
            sync.dma_start(out=st[g % 3][:, :2 * Q],
                           in_=qstr[g % B_PER][:, :2 * Q]).then_inc(sINA, 16)
            sync.dma_start(out=st[g % 3][:, 2 * Q:4 * Q],
                           in_=qstr[g % B_PER][:, 2 * Q:4 * Q]).then_inc(sINC, 16)
            sync.dma_start(out=st[g % 3][:, 4 * Q:],
                           in_=qstr[g % B_PER][:, 4 * Q:]).then_inc(sINB, 16)

        @block.sync
        def _(sync):
            sync.dma_start(out=kc[:], in_=kcol).then_inc(sINA, 16)
            for g in range(min(3, GTOT)):
                load_slot(sync, g)
            for K in range(NTOT):
                rep, k = divmod(K, NK)
                j, i = slabs[k]
                gslot = rep * B_PER + j
                if k == first_slab[j] and 3 <= gslot + 2 < GTOT:
                    # prefetch slot gslot+2 into the buffer slot gslot-1 used;
                    # its consumers finished around slot gslot's first slab.
                    gp = gslot - 1
                    Kp = glast(gp) + 1
                    sync.wait_ge(sDVEa, Kp)
                    sync.wait_ge(sABS, Kp)
                    sync.wait_ge(pWE, Kp)
                    load_slot(sync, gslot + 2)
                sync.wait_ge(pS12, K + 1)
                sync.dma_start(out=cout[k, 0], in_=tl["s1c"][K % 2][:]) \
                    .then_inc(sSTORE, 16)
                sync.wait_ge(sG, K + 1)
                sync.dma_start(out=cout[k, 1], in_=tl["g"][K % 2][:]) \
                    .then_inc(sSTORE, 16)

        @block.vector
        def _(v):
            cd = v._custom_dve

            def kcap(k, c):
                return kc[:, k * NKC + c:k * NKC + c + 1]

            def A(K):
                rep, k = divmod(K, NK)
                j, i = slabs[k]
                P = K % 2
                P3 = K % 3
                gslot = rep * B_PER + j
                if k == first_slab[j] or K < 2:
                    v.wait_ge(sINA, 16 * (gslot + 2))
                if K >= 2:
                    v.wait_ge(sTWTH, K - 1)   # tw/th(K-2) consumed wd/hd(K-2)
                if K >= 3:
                    v.wait_ge(sR1, K - 2)     # r1(K-3) consumed tuU(K-3)
                cd(ops["BHM_IDIFFC"], out=tl["wd"][P][:], in0=S(gslot, S_CX),
                   in1=S(gslot, S_W), s0=kcap(k, K_X1), s1=kcap(k, K_X0),
                   imm2=0.5).then_inc(sWD, 1)
                if k == first_slab[j] or K < 2:
                    v.wait_ge(sINC, 16 * (gslot + 1))
                cd(ops["BHM_IDIFFC"], out=tl["hd"][P][:], in0=S(gslot, S_CY),
                   in1=S(gslot, S_H), s0=kcap(k, K_Y1), s1=kcap(k, K_Y0),
                   imm2=0.5).then_inc(sWH, 1)
                cd(ops["BHM_RELUMULN"], out=tl["inter2"][P3][:],
                   in0=tl["wd"][P][:], in1=tl["hd"][P][:], imm2=-2.0)
                v.tensor_scalar(tl["tuU"][P3][:], tl["inter2"][P3][:], 0.5,
                                kcap(k, K_A2), op0=alu.mult, op1=alu.add)
                if k == first_slab[j] or K < 2:
                    v.wait_ge(sINB, 16 * (gslot + 1))
                v.tensor_tensor(tl["tuU"][P3][:], tl["tuU"][P3][:],
                                S(gslot, S_A1), op=alu.add).then_inc(sDVEa, 1)

            def C1(K):
                rep, k = divmod(K, NK)
                P = K % 2
                v.wait_ge(pWE, K + 1)
                if K >= 2:
                    v.wait_ge(pS12, K - 1)    # Pool s12(K-2) consumed s2(K-2)
                if K >= 2:
                    v.wait_ge(sR2, K - 1)     # r2(K-2) consumed areae(K-2)
                v.tensor_tensor(tl["areae"][P][:], tl["we"][P][:],
                                tl["he"][P][:], op=alu.mult).then_inc(sAREA, 1)
                v.wait_ge(sABS, K + 1)
                if K >= 2:
                    v.wait_ge(sSTORE, 32 * (K - 1))  # slab K-2 fully stored
                v.tensor_tensor(tl["s1c"][P][:], tl["acx"][P][:],
                                tl["acy"][P][:], op=alu.add)
                v.tensor_tensor(tl["s2"][P][:], tl["aw"][P][:], tl["ah"][P][:],
                                op=alu.add).then_inc(sS12, 1)

            def C2(K):
                P = K % 2
                P3 = K % 3
                v.wait_ge(sR1, K + 1)
                v.wait_ge(sR2, K + 1)
                if K >= 2:
                    v.wait_ge(sSTORE, 32 * (K - 1))  # g(K-2) stored
                v.tensor_tensor(tl["p1"][P][:], tl["inter2"][P3][:],
                                tl["r1"][P][:], op=alu.mult)
                v.tensor_tensor(tl["p2"][P][:], tl["tuU"][P3][:], tl["r2"][P][:],
                                op=alu.mult)
                v.tensor_tensor(tl["g"][P][:], tl["p1"][P][:], tl["p2"][P][:],
                                op=alu.add).then_inc(sG, 1)

            for K in range(NTOT):
                A(K)
                if K >= 2:
                    C2(K - 2)
                if K >= 1:
                    C1(K - 1)
            C1(NTOT - 1)
            C2(NTOT - 2)
            C2(NTOT - 1)

        @block.scalar
        def _(a):
            def kcap(k, c):
                return kc[:, k * NKC + c:k * NKC + c + 1]

            def act_r2(m):
                # r2(m) = -2/areae(m); emitted one slab late so DVE's C2(m)
                # never waits on it. r2 reuses we(m)'s buffer: the sAREA wait
                # also guarantees areae = we*he is done reading it.
                Pm = m % 2
                a.wait_ge(sAREA, m + 1)
                a.add_instruction(mybir.InstActivation(
                    name=nc.get_next_instruction_name(), func=AFT.Reciprocal,
                    ins=[a.lower_ap(tl["areae"][Pm][:]),
                         mybir.ImmediateValue(dtype=f32, value=0.0),
                         mybir.ImmediateValue(dtype=f32, value=-0.5),
                         mybir.ImmediateValue(dtype=f32, value=0.0)],
                    outs=[a.lower_ap(tl["r2"][Pm][:])])).then_inc(sR2, 1)

            for K in range(NTOT):
                rep, k = divmod(K, NK)
                j, i = slabs[k]
                P = K % 2
                P3 = K % 3
                gslot = rep * B_PER + j
                a.wait_ge(sWD, K + 1)
                if K >= 2:
                    a.wait_ge(pWE, K - 1)     # we/he(K-2) consumed tw/th(K-2)
                a.activation(tl["tw"][P][:], tl["wd"][P][:], AFT.Identity,
                             bias=kcap(k, K_WT), scale=-1.0).then_inc(sTW, 1)
                a.wait_ge(sWH, K + 1)
                a.activation(tl["th"][P][:], tl["hd"][P][:], AFT.Identity,
                             bias=kcap(k, K_HT),
                             scale=-1.0).then_inc(sTWTH, 1)
                a.wait_ge(sDVEa, K + 1)
                if K >= 2:
                    # C2(K-2) done: frees r1(K-2) and the p1/p2/g aliases of
                    # the abs tiles written below.
                    a.wait_ge(sG, K - 1)
                a.add_instruction(mybir.InstActivation(
                    name=nc.get_next_instruction_name(), func=AFT.Reciprocal,
                    ins=[a.lower_ap(tl["tuU"][P3][:]),
                         mybir.ImmediateValue(dtype=f32, value=0.0),
                         mybir.ImmediateValue(dtype=f32, value=1.0),
                         mybir.ImmediateValue(dtype=f32, value=0.0)],
                    outs=[a.lower_ap(tl["r1"][P][:])])).then_inc(sR1, 1)
                if K >= 2:
                    a.wait_ge(sS12, K - 1)    # s1/s2(K-2) consumed abs(K-2)
                a.activation(tl["acx"][P][:], S(gslot, S_CX), AFT.Abs,
                             bias=kcap(k, K_BCX), scale=5.0)
                a.activation(tl["acy"][P][:], S(gslot, S_CY), AFT.Abs,
                             bias=kcap(k, K_BCY), scale=5.0)
                a.activation(tl["aw"][P][:], S(gslot, S_W), AFT.Abs,
                             bias=kcap(k, K_BW), scale=5.0)
                a.activation(tl["ah"][P][:], S(gslot, S_H), AFT.Abs,
                             bias=kcap(k, K_BH), scale=5.0).then_inc(sABS, 1)
                if K >= 1:
                    act_r2(K - 1)
            act_r2(NTOT - 1)

        @block.gpsimd
        def _(g):
            def pool_we(m):
                rep, k = divmod(m, NK)
                j, i = slabs[k]
                Pm = m % 2
                gslot = rep * B_PER + j
                g.wait_ge(sTW, m + 1)
                if m >= 2:
                    g.wait_ge(sAREA, m - 1)   # areae(m-2) consumed we/he(m-2)
                g.tensor_tensor(tl["we"][Pm][:], tl["tw"][Pm][:],
                                S(gslot, S_W), op=alu.add)
                g.wait_ge(sTWTH, m + 1)
                g.tensor_tensor(tl["he"][Pm][:], tl["th"][Pm][:],
                                S(gslot, S_H), op=alu.add).then_inc(pWE, 1)

            pool_we(0)
            for K in range(NTOT):
                rep, k = divmod(K, NK)
                j, i = slabs[k]
                P = K % 2
                gslot = rep * B_PER + j
                if K + 1 < NTOT:
                    pool_we(K + 1)
                g.wait_ge(sS12, K + 1)
                g.tensor_tensor(tl["s1c"][P][:], tl["s1c"][P][:], tl["s2"][P][:],
                                op=alu.add).then_inc(pS12, 1)

    mybir.codegen_inst_isa_subclasses(nc)
    return nc


def _host_prep(pred_logits, pred_boxes, boxes_padded, num_boxes, slots, ntiles):
    import ml_dtypes
    bf16 = ml_dtypes.bfloat16

    pl = np.asarray(pred_logits, np.float64)[..., 0]
    pb = np.asarray(pred_boxes, np.float64)
    tb = np.asarray(boxes_padded, np.float64)

    cx, cy, w, h = pb[..., 0], pb[..., 1], pb[..., 2], pb[..., 3]
    a1 = w * h
    p = 1.0 / (1.0 + np.exp(-pl))
    log_p = -np.log1p(np.exp(-pl))
    log_1mp = -np.log1p(np.exp(pl))
    cc = -0.25 * (1.0 - p) ** 2 * log_p + 0.75 * p ** 2 * log_1mp
    cc2 = (2.0 * cc + 2.0).astype(np.float32)               # host-side add
    qvals = np.stack([cx, w, cy, h, a1], axis=1)            # [B, NSTR, Q]

    tcx, tcy, tw, th = tb[..., 0], tb[..., 1], tb[..., 2], tb[..., 3]
    tx0, tx1 = tcx - 0.5 * tw, tcx + 0.5 * tw
    ty0, ty1 = tcy - 0.5 * th, tcy + 0.5 * th
    a2 = tw * th
    kvals = np.stack([tx0, tx1, ty0, ty1, -5.0 * tcx, -5.0 * tcy,
                      -5.0 * tw, -5.0 * th, tw, th, a2], axis=1)  # [B, NKC, T]
    kpad = np.array([0.0, 1.0, 0.0, 1.0, -2.5, -2.5, -5.0, -5.0, 1.0, 1.0, 1.0])

    slabs = [(j, i) for j in range(B_PER) for i in range(ntiles[j])]
    NK = len(slabs)
    in_maps = []
    for c in range(N_CORES):
        qs = np.empty((B_PER, TP, NSTR * Q), dtype=bf16)
        for j in range(B_PER):
            b = int(slots[j][c])
            qs[j] = np.broadcast_to(
                qvals[b].astype(bf16).reshape(1, NSTR * Q), (TP, NSTR * Q))
        kc = np.empty((TP, NK * NKC), np.float32)
        for k, (j, i) in enumerate(slabs):
            b = int(slots[j][c])
            t0 = i * TP
            nrow = min(TP, T - t0)
            kc[:nrow, k * NKC:(k + 1) * NKC] = kvals[b, :, t0:t0 + nrow].T
            if nrow < TP:
                kc[nrow:, k * NKC:(k + 1) * NKC] = kpad[None, :]
        in_maps.append({"qstr": qs, "kcol": kc})
    return in_maps, cc2


def kernel(pred_logits, pred_boxes, boxes_padded, num_boxes):
    global LAST_RESULTS
    from concourse.bass_utils import run_bass_kernel_spmd

    slots, ntiles = _plan(num_boxes)
    in_maps, cc2 = _host_prep(pred_logits, pred_boxes, boxes_padded, num_boxes,
                              slots, ntiles)
    nc = _PROG_CACHE.get(ntiles)
    if nc is None:
        nc = _build_program(ntiles)
        _PROG_CACHE[ntiles] = nc
    res = None
    for attempt in range(3):
        try:
            res = run_bass_kernel_spmd(nc, in_maps, list(range(N_CORES)))
            break
        except Exception:
            # transient NRT device wedges resolve on re-execution
            if attempt == 2:
                raise
    LAST_RESULTS = res

    nb = np.asarray(num_boxes).astype(np.int64)
    slabs = [(j, i) for j in range(B_PER) for i in range(ntiles[j])]
    out = np.empty((B, Q, T), np.float32)
    out[:] = INVALID
    for c in range(N_CORES):
        slab_arr = np.asarray(res.results[c]["C"]).astype(np.float32)
        for k, (j, i) in enumerate(slabs):
            b = int(slots[j][c])
            t0 = i * TP
            nrow = min(TP, T - t0)
            # C = 5*L1 + (-2*giou part) + per-query class cost, final adds
            # in f32 on the host
            out[b, :, t0:t0 + nrow] = \
                (slab_arr[k, 0, :nrow] + slab_arr[k, 1, :nrow]).T \
                + cc2[b][:, None]
    for b in range(B):
        out[b, :, nb[b]:] = INVALID
    return out


# revision 8
# speedup vs baseline: 1.0497x; 1.0497x over previous
"""Trainium2 Bass kernel v2: BinaryHungarianMatcherV2 cost-matrix build.

C[b,q,t] = 5*L1(pred_box, tgt_box) + 2*focal_class(q) + 2 - 2*giou,
invalid targets (t >= num_boxes[b]) fixed to 1e9 on the host.

Layout: t on the partition axis, q on the free axis (1800 wide). Per core
4 batch slots (batch dim sharded over 8 cores, slots sorted by num_boxes
so cores sharing the SPMD program do similar work); per slot
ceil(W/128) t-slabs of [128 x 1800]. Per-target values ride as
per-partition scalar columns; per-query values are bf16 streams
replicated across partitions (one DMA per slot, triple-buffered so
prefetch never waits on consumers).

Work is split across three engines per slab (bf16, DVE 2x mode for
tensor_tensor), software-pipelined with the DVE running phases
A(K) | C2(K-2) | C1(K-1) and the Pool running we/he one slab ahead of
its u1/out tail so no engine's in-order queue blocks another's inputs:
  DVE  A : wd/hd corner-overlap customs, -2*inter custom,
           tensor_scalar (a2 - inter), union add
  ACT    : tw/th (we/he partials), r1 = 1/union, 4x Abs (L1 terms),
           r2(K-1) = -2/areae (one slab late by construction)
  Pool   : we/he adds (slab K+1), u1 = s12 + cc2, out = u1 + g
  DVE C1 : areae = we*he, s1/s2/s12 abs-sums
  DVE C2 : p1 = inter2*r1, p2 = union*r2, g = p1+p2
Intermediate tiles alias where lifetimes are disjoint (p1/p2/g reuse the
abs tiles, areae/s2 reuse wd/hd, r2 reuses we).
"""

import os
from contextlib import ExitStack

import numpy as np

B, Q, T = 32, 1800, 500
N_CORES = 8
B_PER = B // N_CORES
TP = 128                       # t-partition tile size
NSTR = 5                       # streams: cx, w, cy, h, a1 (wd's pair first)
S_CX, S_W, S_CY, S_H, S_A1 = range(NSTR)
NKC = 11                       # per-slab scalar columns
K_X0, K_X1, K_Y0, K_Y1, K_BCX, K_BCY, K_BW, K_BH, K_WT, K_HT, K_A2 = range(NKC)

INVALID = 1.0e9

_OPS = None
_PROG_CACHE = {}
LAST_RESULTS = None


def _get_ops():
    """Register custom DVE ops (idempotent)."""
    global _OPS
    if _OPS is not None:
        return _OPS
    from concourse import dve_ops
    from concourse.dve_ops import DveOp
    from concourse.dve_spec import Spec, Src0, Src1, C0, C1, C2, relu, maxx, minn, lower
    from concourse.dve_uop import DveOpSpec

    def reg(name, spec):
        for op in dve_ops.OPS:
            if op.name == name:
                return op
        row = max(dve_ops._SUB_OPCODE_FOR_NAME.values()) + 1
        assert row < 0x20, "custom-DVE opcode rows exhausted"
        dve_ops._SUB_OPCODE_FOR_NAME[name] = row
        shas = {}
        for ver in ("v3", "v4"):
            s = DveOpSpec(name=name, opcode=row, uops=lower(spec, ver=ver),
                          rd1_en=dve_ops.has_src1(spec))
            shas[ver] = s.sha(ver)
        op = DveOp(name, spec, subdim=False, uops_sha=shas)
        dve_ops.OPS.append(op)
        dve_ops.CUSTOM_DVE_SPECS[name] = spec
        return op

    _OPS = {
        # wd = min(cx + 0.5*w, x1t) - max(cx - 0.5*w, x0t); C0=x1t, C1=x0t, C2=0.5
        "BHM_IDIFFC": reg("BHM_IDIFFC", Spec(
            body=minn(Src0 + Src1 * C2, C0) - maxx(Src0 - Src1 * C2, C1),
            reference=lambda in0, in1, s0, s1, imm2:
                np.minimum(in0 + in1 * imm2, s0) - np.maximum(in0 - in1 * imm2, s1))),
        # inter2 = relu(wd)*relu(hd)*C2 (C2 = -2)
        "BHM_RELUMULN": reg("BHM_RELUMULN", Spec(
            body=(relu(Src0) * relu(Src1)) * C2,
            reference=lambda in0, in1, s0, s1, imm2:
                np.maximum(in0, 0) * np.maximum(in1, 0) * imm2)),
    }
    return _OPS


def _plan(num_boxes):
    """Sort batches by num_boxes; slot j holds sorted[8j:8j+8] (one per core).
    Returns (slots[B_PER][N_CORES], ntiles tuple)."""
    nb = np.asarray(num_boxes).astype(np.int64)
    order = np.argsort(nb, kind="stable")
    slots = order.reshape(B_PER, N_CORES)
    ntiles = tuple(int(-(-int(nb[slots[j]].max()) // TP)) for j in range(B_PER))
    return slots, ntiles


def _build_program(ntiles):
    import concourse.bass as bass
    from concourse import mybir

    ops = _get_ops()
    f32 = mybir.dt.float32
    bf16 = mybir.dt.bfloat16
    alu = mybir.AluOpType
    AFT = mybir.ActivationFunctionType
    nc = bass.Bass("TRN2")

    slabs = [(j, i) for j in range(B_PER) for i in range(ntiles[j])]
    NK = len(slabs)
    REPEAT = int(os.environ.get("BHM_REPEAT", "1"))
    NTOT = NK * REPEAT
    GTOT = B_PER * REPEAT
    first_slab = {}
    last_slab = {}
    for k, (j, i) in enumerate(slabs):
        first_slab.setdefault(j, k)
        last_slab[j] = k

    def glast(g):
        """Global K index of the last slab of global slot g."""
        return (g // B_PER) * NK + last_slab[g % B_PER]

    qstr = nc.dram_tensor("qstr", [B_PER, TP, NSTR * Q], bf16,
                          kind="ExternalInput").ap()
    kcol = nc.dram_tensor("kcol", [TP, NK * NKC], f32, kind="ExternalInput").ap()
    # four part-results per slab; the host sums them (plus the per-query
    # class cost, which never has to touch the device) during assembly.
    cout = nc.dram_tensor("C", [NK, 4, TP, Q], bf16, kind="ExternalOutput").ap()

    with ExitStack() as ctx:
        st = [ctx.enter_context(nc.sbuf_tensor(f"st_{p}", [TP, NSTR * Q], bf16))
              for p in range(3)]
        kc = ctx.enter_context(nc.sbuf_tensor("kc", [TP, NK * NKC], f32))

        tnames = ["wd", "hd", "acx", "acy", "aw", "ah", "s1c", "s2", "tw",
                  "th", "we", "he", "r1", "r2", "areae", "p1", "p2"]
        tl = {n: [ctx.enter_context(nc.sbuf_tensor(f"t_{n}_{p}", [TP, Q], bf16))
                  for p in range(2)] for n in tnames}
        for n in ("inter2", "tuU"):
            tl[n] = [ctx.enter_context(nc.sbuf_tensor(f"t_{n}_{p}", [TP, Q], bf16))
                     for p in range(3)]


        sINA = ctx.enter_context(nc.semaphore("sINA"))     # kcol + cx/w streams
        sINC = ctx.enter_context(nc.semaphore("sINC"))     # cy/h streams
        sINB = ctx.enter_context(nc.semaphore("sINB"))     # a1 streams
        sWD = ctx.enter_context(nc.semaphore("sWD"))       # DVE wd done
        sWH = ctx.enter_context(nc.semaphore("sWH"))       # DVE wd,hd done
        sDVEa = ctx.enter_context(nc.semaphore("sDVEa"))   # DVE union done
        sS12 = ctx.enter_context(nc.semaphore("sS12"))     # DVE s12 done
        sAREA = ctx.enter_context(nc.semaphore("sAREA"))   # DVE areae done
        sG = ctx.enter_context(nc.semaphore("sG"))         # DVE g done
        sTW = ctx.enter_context(nc.semaphore("sTW"))       # ACT tw done
        sTWTH = ctx.enter_context(nc.semaphore("sTWTH"))   # ACT tw,th done
        sABS = ctx.enter_context(nc.semaphore("sABS"))     # ACT abs group done
        sR1 = ctx.enter_context(nc.semaphore("sR1"))
        sR2 = ctx.enter_context(nc.semaphore("sR2"))
        pWE = ctx.enter_context(nc.semaphore("pWE"))       # Pool we,he done
        pS12 = ctx.enter_context(nc.semaphore("pS12"))     # Pool s12 done
        sSTA = ctx.enter_context(nc.semaphore("sSTA"))     # abs-part stores
        sSTP = ctx.enter_context(nc.semaphore("sSTP"))     # p1/p2 stores
        block = ctx.enter_context(nc.Block())

        def S(g, s):
            return st[g % 3][:, s * Q:(s + 1) * Q]

        def load_slot(sync, g):
            # wd's pair (cx,w) first, then hd's (cy,h), then a1, so the DVE
            # starts as soon as the first 0.9 MB lands
            sync.dma_start(out=st[g % 3][:, :2 * Q],
                           in_=qstr[g % B_PER][:, :2 * Q]).then_inc(sINA, 16)
            sync.dma_start(out=st[g % 3][:, 2 * Q:4 * Q],
                           in_=qstr[g % B_PER][:, 2 * Q:4 * Q]).then_inc(sINC, 16)
            sync.dma_start(out=st[g % 3][:, 4 * Q:],
                           in_=qstr[g % B_PER][:, 4 * Q:]).then_inc(sINB, 16)

        @block.sync
        def _(sync):
            sync.dma_start(out=kc[:], in_=kcol).then_inc(sINA, 16)
            for g in range(min(3, GTOT)):
                load_slot(sync, g)
            for K in range(NTOT):
                rep, k = divmod(K, NK)
                j, i = slabs[k]
                gslot = rep * B_PER + j
                if k == first_slab[j] and 3 <= gslot + 2 < GTOT:
                    # prefetch slot gslot+2 into the buffer slot gslot-1 used;
                    # its consumers finished around slot gslot's first slab.
                    gp = gslot - 1
                    Kp = glast(gp) + 1
                    sync.wait_ge(sDVEa, Kp)
                    sync.wait_ge(sABS, Kp)
                    sync.wait_ge(pWE, Kp)
                    load_slot(sync, gslot + 2)
                sync.wait_ge(sS12, K + 1)
                sync.dma_start(out=cout[k, 0], in_=tl["s1c"][K % 2][:]) \
                    .then_inc(sSTA, 16)
                sync.wait_ge(pS12, K + 1)
                sync.dma_start(out=cout[k, 1], in_=tl["s2"][K % 2][:]) \
                    .then_inc(sSTA, 16)
                sync.wait_ge(sG, K + 1)
                sync.dma_start(out=cout[k, 2], in_=tl["p1"][K % 2][:]) \
                    .then_inc(sSTP, 16)
                sync.dma_start(out=cout[k, 3], in_=tl["p2"][K % 2][:]) \
                    .then_inc(sSTP, 16)

        @block.vector
        def _(v):
            cd = v._custom_dve

            def kcap(k, c):
                return kc[:, k * NKC + c:k * NKC + c + 1]

            def A(K):
                rep, k = divmod(K, NK)
                j, i = slabs[k]
                P = K % 2
                P3 = K % 3
                gslot = rep * B_PER + j
                if k == first_slab[j] or K < 2:
                    v.wait_ge(sINA, 16 * (gslot + 2))
                if K >= 2:
                    v.wait_ge(pWE, K - 1)     # we/he(K-2) consumed tw/th(K-2)
                if K >= 3:
                    v.wait_ge(sR1, K - 2)     # r1(K-3) consumed tuU(K-3)
                cd(ops["BHM_IDIFFC"], out=tl["wd"][P][:], in0=S(gslot, S_CX),
                   in1=S(gslot, S_W), s0=kcap(k, K_X1), s1=kcap(k, K_X0),
                   imm2=0.5).then_inc(sWD, 1)
                if k == first_slab[j] or K < 2:
                    v.wait_ge(sINC, 16 * (gslot + 1))
                cd(ops["BHM_IDIFFC"], out=tl["hd"][P][:], in0=S(gslot, S_CY),
                   in1=S(gslot, S_H), s0=kcap(k, K_Y1), s1=kcap(k, K_Y0),
                   imm2=0.5)
                # th = Kht - hd on the 4x tensor_scalar path (off the ACT)
                v.tensor_scalar(tl["th"][P][:], tl["hd"][P][:], kcap(k, K_HT),
                                -1.0, op0=alu.subtract,
                                op1=alu.mult).then_inc(sWH, 1)
                cd(ops["BHM_RELUMULN"], out=tl["inter2"][P3][:],
                   in0=tl["wd"][P][:], in1=tl["hd"][P][:], imm2=-2.0)
                v.tensor_scalar(tl["tuU"][P3][:], tl["inter2"][P3][:], 0.5,
                                kcap(k, K_A2), op0=alu.mult, op1=alu.add)
                if k == first_slab[j] or K < 2:
                    v.wait_ge(sINB, 16 * (gslot + 1))
                v.tensor_tensor(tl["tuU"][P3][:], tl["tuU"][P3][:],
                                S(gslot, S_A1), op=alu.add).then_inc(sDVEa, 1)

            def C1(K):
                rep, k = divmod(K, NK)
                P = K % 2
                v.wait_ge(pWE, K + 1)
                if K >= 2:
                    v.wait_ge(sR2, K - 1)     # r2(K-2) consumed areae(K-2)
                v.tensor_tensor(tl["areae"][P][:], tl["we"][P][:],
                                tl["he"][P][:], op=alu.mult).then_inc(sAREA, 1)
                v.wait_ge(sABS, K + 1)
                if K >= 2:
                    v.wait_ge(sSTA, 32 * (K - 1))    # s1(K-2) stored
                v.tensor_tensor(tl["s1c"][P][:], tl["acx"][P][:],
                                tl["acy"][P][:], op=alu.add).then_inc(sS12, 1)


            def C2(K):
                P = K % 2
                P3 = K % 3
                v.wait_ge(sR1, K + 1)
                v.wait_ge(sR2, K + 1)
                if K >= 2:
                    v.wait_ge(sSTP, 32 * (K - 1))    # p1/p2(K-2) stored
                v.tensor_tensor(tl["p1"][P][:], tl["inter2"][P3][:],
                                tl["r1"][P][:], op=alu.mult)
                v.tensor_tensor(tl["p2"][P][:], tl["tuU"][P3][:], tl["r2"][P][:],
                                op=alu.mult).then_inc(sG, 1)

            for K in range(NTOT):
                A(K)
                if K >= 2:
                    C2(K - 2)
                if K >= 1:
                    C1(K - 1)
            C1(NTOT - 1)
            C2(NTOT - 2)
            C2(NTOT - 1)

        @block.scalar
        def _(a):
            def kcap(k, c):
                return kc[:, k * NKC + c:k * NKC + c + 1]

            def act_r2(m):
                # r2(m) = -2/areae(m); emitted one slab late so DVE's C2(m)
                # never waits on it. r2 reuses we(m)'s buffer: the sAREA wait
                # also guarantees areae = we*he is done reading it.
                Pm = m % 2
                a.wait_ge(sAREA, m + 1)
                a.add_instruction(mybir.InstActivation(
                    name=nc.get_next_instruction_name(), func=AFT.Reciprocal,
                    ins=[a.lower_ap(tl["areae"][Pm][:]),
                         mybir.ImmediateValue(dtype=f32, value=0.0),
                         mybir.ImmediateValue(dtype=f32, value=-0.5),
                         mybir.ImmediateValue(dtype=f32, value=0.0)],
                    outs=[a.lower_ap(tl["r2"][Pm][:])])).then_inc(sR2, 1)

            for K in range(NTOT):
                rep, k = divmod(K, NK)
                j, i = slabs[k]
                P = K % 2
                P3 = K % 3
                gslot = rep * B_PER + j

                a.wait_ge(sWD, K + 1)
                if K >= 2:
                    a.wait_ge(pWE, K - 1)     # we(K-2) consumed tw(K-2)
                a.activation(tl["tw"][P][:], tl["wd"][P][:], AFT.Identity,
                             bias=kcap(k, K_WT), scale=-1.0).then_inc(sTW, 1)
                a.wait_ge(sDVEa, K + 1)
                if K >= 2:
                    a.wait_ge(sG, K - 1)
                a.add_instruction(mybir.InstActivation(
                    name=nc.get_next_instruction_name(), func=AFT.Reciprocal,
                    ins=[a.lower_ap(tl["tuU"][P3][:]),
                         mybir.ImmediateValue(dtype=f32, value=0.0),
                         mybir.ImmediateValue(dtype=f32, value=1.0),
                         mybir.ImmediateValue(dtype=f32, value=0.0)],
                    outs=[a.lower_ap(tl["r1"][P][:])])).then_inc(sR1, 1)
                if K >= 2:
                    a.wait_ge(sS12, K - 1)    # s1(K-2) consumed acx/acy(K-2)
                    a.wait_ge(pS12, K - 1)    # s2(K-2) consumed aw/ah(K-2)
                a.activation(tl["acx"][P][:], S(gslot, S_CX), AFT.Abs,
                             bias=kcap(k, K_BCX), scale=5.0)
                a.activation(tl["acy"][P][:], S(gslot, S_CY), AFT.Abs,
                             bias=kcap(k, K_BCY), scale=5.0)
                a.activation(tl["aw"][P][:], S(gslot, S_W), AFT.Abs,
                             bias=kcap(k, K_BW), scale=5.0)
                a.activation(tl["ah"][P][:], S(gslot, S_H), AFT.Abs,
                             bias=kcap(k, K_BH), scale=5.0).then_inc(sABS, 1)
                if K >= 1:
                    act_r2(K - 1)
            act_r2(NTOT - 1)

        @block.gpsimd
        def _(g):
            def pool_we(m):
                rep, k = divmod(m, NK)
                j, i = slabs[k]
                Pm = m % 2
                gslot = rep * B_PER + j
                g.wait_ge(sTW, m + 1)
                if m >= 2:
                    g.wait_ge(sAREA, m - 1)   # areae(m-2) consumed we/he(m-2)
                g.tensor_tensor(tl["we"][Pm][:], tl["tw"][Pm][:],
                                S(gslot, S_W), op=alu.add)
                g.wait_ge(sWH, m + 1)
                g.tensor_tensor(tl["he"][Pm][:], tl["th"][Pm][:],
                                S(gslot, S_H), op=alu.add).then_inc(pWE, 1)

            pool_we(0)
            for K in range(NTOT):
                rep, k = divmod(K, NK)
                j, i = slabs[k]
                P = K % 2
                gslot = rep * B_PER + j
                if K + 1 < NTOT:
                    pool_we(K + 1)
                g.wait_ge(sABS, K + 1)
                if K >= 2:
                    g.wait_ge(sSTA, 32 * (K - 1))    # s2(K-2) stored
                g.tensor_tensor(tl["s2"][P][:], tl["aw"][P][:], tl["ah"][P][:],
                                op=alu.add).then_inc(pS12, 1)

    mybir.codegen_inst_isa_subclasses(nc)
    return nc


def _host_prep(pred_logits, pred_boxes, boxes_padded, num_boxes, slots, ntiles):
    import ml_dtypes
    bf16 = ml_dtypes.bfloat16

    pl = np.asarray(pred_logits, np.float64)[..., 0]
    pb = np.asarray(pred_boxes, np.float64)
    tb = np.asarray(boxes_padded, np.float64)

    cx, cy, w, h = pb[..., 0], pb[..., 1], pb[..., 2], pb[..., 3]
    a1 = w * h
    p = 1.0 / (1.0 + np.exp(-pl))
    log_p = -np.log1p(np.exp(-pl))
    log_1mp = -np.log1p(np.exp(pl))
    cc = -0.25 * (1.0 - p) ** 2 * log_p + 0.75 * p ** 2 * log_1mp
    cc2 = (2.0 * cc + 2.0).astype(np.float32)               # host-side add
    qvals = np.stack([cx, w, cy, h, a1], axis=1)            # [B, NSTR, Q]

    tcx, tcy, tw, th = tb[..., 0], tb[..., 1], tb[..., 2], tb[..., 3]
    tx0, tx1 = tcx - 0.5 * tw, tcx + 0.5 * tw
    ty0, ty1 = tcy - 0.5 * th, tcy + 0.5 * th
    a2 = tw * th
    kvals = np.stack([tx0, tx1, ty0, ty1, -5.0 * tcx, -5.0 * tcy,
                      -5.0 * tw, -5.0 * th, tw, th, a2], axis=1)  # [B, NKC, T]
    kpad = np.array([0.0, 1.0, 0.0, 1.0, -2.5, -2.5, -5.0, -5.0, 1.0, 1.0, 1.0])

    slabs = [(j, i) for j in range(B_PER) for i in range(ntiles[j])]
    NK = len(slabs)
    in_maps = []
    for c in range(N_CORES):
        qs = np.empty((B_PER, TP, NSTR * Q), dtype=bf16)
        for j in range(B_PER):
            b = int(slots[j][c])
            qs[j] = np.broadcast_to(
                qvals[b].astype(bf16).reshape(1, NSTR * Q), (TP, NSTR * Q))
        kc = np.empty((TP, NK * NKC), np.float32)
        for k, (j, i) in enumerate(slabs):
            b = int(slots[j][c])
            t0 = i * TP
            nrow = min(TP, T - t0)
            kc[:nrow, k * NKC:(k + 1) * NKC] = kvals[b, :, t0:t0 + nrow].T
            if nrow < TP:
                kc[nrow:, k * NKC:(k + 1) * NKC] = kpad[None, :]
        in_maps.append({"qstr": qs, "kcol": kc})
    return in_maps, cc2


def kernel(pred_logits, pred_boxes, boxes_padded, num_boxes):
    global LAST_RESULTS
    from concourse.bass_utils import run_bass_kernel_spmd

    slots, ntiles = _plan(num_boxes)
    in_maps, cc2 = _host_prep(pred_logits, pred_boxes, boxes_padded, num_boxes,
                              slots, ntiles)
    nc = _PROG_CACHE.get(ntiles)
    if nc is None:
        nc = _build_program(ntiles)
        _PROG_CACHE[ntiles] = nc
    res = None
    for attempt in range(3):
        try:
            res = run_bass_kernel_spmd(nc, in_maps, list(range(N_CORES)))
            break
        except Exception:
            # transient NRT device wedges resolve on re-execution
            if attempt == 2:
                raise
    LAST_RESULTS = res

    nb = np.asarray(num_boxes).astype(np.int64)
    slabs = [(j, i) for j in range(B_PER) for i in range(ntiles[j])]
    out = np.empty((B, Q, T), np.float32)
    out[:] = INVALID
    for c in range(N_CORES):
        slab_arr = np.asarray(res.results[c]["C"]).astype(np.float32)
        for k, (j, i) in enumerate(slabs):
            b = int(slots[j][c])
            t0 = i * TP
            nrow = min(TP, T - t0)
            # C = 5*L1 + (-2*giou part) + per-query class cost, final adds
            # in f32 on the host
            out[b, :, t0:t0 + nrow] = \
                slab_arr[k, :, :nrow].sum(axis=0).T + cc2[b][:, None]
    for b in range(B):
        out[b, :, nb[b]:] = INVALID
    return out
